# revision 2
# baseline (speedup 1.0000x reference)
"""AtomicConvLayer (GNN message passing) on 8 Trainium2 NeuronCores.

Reference computation (per atom i, neighbors j = nbr[i, 0..31]):
    h_ij   = relu(x_i @ W1a + x_j @ W1b + b1)         (msg_W1 split in two)
    agg_i  = sum_j (h_ij @ W2 + b2)
    u_i    = relu(x_i @ U1a + agg_i @ U1b + bu1)
    out_i  = relu(x_i + u_i @ UW2 + bu2)

Algebraic restructuring used here (exact in exact arithmetic):
    B      = X @ W1b                (25600x128 table, computed per core)
    A_i    = x_i @ W1a + b1
    Hsum_i = sum_j relu(A_i + B[nbr_ij])             <- only gather B rows
    u_i    = relu(x_i @ U1a + Hsum_i @ (W2 @ U1b) + (bu1 + 32*b2 @ U1b))
    out_i  = relu(x_i + u_i @ UW2 + bu2)

Sharding: data-parallel over atoms. Each core owns 3200 consecutive atoms
(25000 padded to 25600), holds the full atom table, computes the full B
table locally (12.8 MB, cheaper than cross-core gathers), then gathers its
own 3200*32 neighbor rows from B with dma_gather.
"""

import sys

sys.path.insert(0, "/opt/trn_rl_repo")

import numpy as np

N_ATOMS = 25000
N_PAD = 25600          # 8 cores x 3200
D = 128
M = 32                 # neighbors per atom
N_CORES = 8
OWN = N_PAD // N_CORES          # 3200 atoms per core
BLOCKS = OWN // 128             # 25 blocks of 128 atoms per core
TILES = N_PAD // 128            # 200 tiles in the full table
LOAD_CHUNK = 16                 # tiles per phase-1 DMA

_CACHE = {}
last_results = None


def _build_nc():
    import concourse.bacc as bacc
    import concourse.mybir as mybir
    import concourse.tile as tile
    from concourse.bass_interp import get_hw_module
    from concourse.masks import make_identity

    f32 = mybir.dt.float32
    nc = bacc.Bacc("TRN2", target_bir_lowering=False, debug=False)

    atoms_d = nc.dram_tensor("atoms", [N_PAD, D], f32, kind="ExternalInput")
    ownx_d = nc.dram_tensor("own_x", [OWN, D], f32, kind="ExternalInput")
    idx_d = nc.dram_tensor("idx16", [128, BLOCKS * 256], mybir.dt.int16, kind="ExternalInput")
    w1a_d = nc.dram_tensor("w1a", [D, D], f32, kind="ExternalInput")
    w1b_d = nc.dram_tensor("w1b", [D, D], f32, kind="ExternalInput")
    b1_d = nc.dram_tensor("b1", [1, D], f32, kind="ExternalInput")
    w2_d = nc.dram_tensor("w2", [D, D], f32, kind="ExternalInput")
    b2c_d = nc.dram_tensor("b2c", [D, 1], f32, kind="ExternalInput")
    u1a_d = nc.dram_tensor("u1a", [D, D], f32, kind="ExternalInput")
    u1b_d = nc.dram_tensor("u1b", [D, D], f32, kind="ExternalInput")
    bu1_d = nc.dram_tensor("bu1", [1, D], f32, kind="ExternalInput")
    uw2_d = nc.dram_tensor("uw2", [D, D], f32, kind="ExternalInput")
    bu2_d = nc.dram_tensor("bu2", [1, D], f32, kind="ExternalInput")
    out_d = nc.dram_tensor("out", [OWN, D], f32, kind="ExternalOutput")

    atoms_v = atoms_d.rearrange("(n p) d -> p n d", p=128)   # [128, 200, 128]
    out_v = out_d.rearrange("(n p) d -> p n d", p=128)       # [128, 25, 128]

    with tile.TileContext(nc) as tc:
        with (
            tc.tile_pool(name="persist", bufs=1) as per,
            tc.tile_pool(name="dram", bufs=1, space="DRAM") as dram,
        ):
            ident = per.tile([128, 128], f32)
            make_identity(nc, ident[:])
            ones_row = per.tile([1, 128], f32)
            nc.gpsimd.memset(ones_row[:], 1.0)

            w1a = per.tile([D, D], f32)
            w1b = per.tile([D, D], f32)
            b1 = per.tile([1, D], f32)
            w2 = per.tile([D, D], f32)
            b2c = per.tile([D, 1], f32)
            u1a = per.tile([D, D], f32)
            u1b = per.tile([D, D], f32)
            bu1 = per.tile([1, D], f32)
            uw2 = per.tile([D, D], f32)
            bu2 = per.tile([1, D], f32)
            idx_sb = per.tile([128, BLOCKS * 256], mybir.dt.int16)
            for sb, d in [(w1a, w1a_d), (w1b, w1b_d), (b1, b1_d), (w2, w2_d),
                          (b2c, b2c_d), (u1a, u1a_d), (u1b, u1b_d), (bu1, bu1_d),
                          (uw2, uw2_d), (bu2, bu2_d), (idx_sb, idx_d)]:
                nc.sync.dma_start(sb[:], d[:])

            x_own = per.tile([128, BLOCKS, D], f32)
            xT_own = per.tile([128, BLOCKS, D], f32)
            a_own = per.tile([128, BLOCKS, D], f32)
            ostage = per.tile([128, BLOCKS, D], f32)
            w2u = per.tile([D, D], f32)
            biasu = per.tile([1, D], f32)

            bdram = dram.tile([N_PAD, D], f32)
            bdram_v = bdram[:].rearrange("(n p) d -> p n d", p=128)

            # ---- weight folds: w2u = W2 @ U1b ; biasu = bu1 + 32*b2 @ U1b
            with tc.tile_pool(name="ps0", bufs=1, space="PSUM") as ps0:
                ps_wt = ps0.tile([128, 128], f32)
                nc.tensor.transpose(ps_wt[:], w2[:], ident[:])
                w2t = per.tile([D, D], f32)
                nc.vector.tensor_copy(w2t[:], ps_wt[:])
                ps_w2u = ps0.tile([128, 128], f32)
                nc.tensor.matmul(ps_w2u[:], w2t[:], u1b[:], start=True, stop=True)
                nc.vector.tensor_copy(w2u[:], ps_w2u[:])

                b2s = per.tile([D, 1], f32)
                nc.vector.tensor_scalar_mul(b2s[:], b2c[:], float(M))
                ps_c = ps0.tile([1, 128], f32)
                nc.tensor.matmul(ps_c[:], b2s[:], u1b[:], start=True, stop=True)
                nc.vector.tensor_tensor(out=biasu[:], in0=ps_c[:], in1=bu1[:],
                                        op=mybir.AluOpType.add)

            # ---- phase 1: B = atoms @ W1b  -> bdram
            with tc.tile_pool(name="p1", bufs=2) as p1, \
                 tc.tile_pool(name="ps1", bufs=2, space="PSUM") as ps1:
                t0 = 0
                while t0 < TILES:
                    k = min(LOAD_CHUNK, TILES - t0)
                    xin = p1.tile([128, LOAD_CHUNK, D], f32, tag="xin")
                    nc.sync.dma_start(xin[:, :k, :], atoms_v[:, t0:t0 + k, :])
                    bstage = p1.tile([128, LOAD_CHUNK, D], f32, tag="bstage")
                    for i in range(k):
                        ps_t = ps1.tile([128, 128], f32, tag="ps_t")
                        nc.tensor.transpose(ps_t[:], xin[:, i, :], ident[:])
                        xt = p1.tile([128, 128], f32, tag="xt")
                        nc.scalar.copy(xt[:], ps_t[:])
                        ps_b = ps1.tile([128, 128], f32, tag="ps_b")
                        nc.tensor.matmul(ps_b[:], xt[:], w1b[:], start=True, stop=True)
                        nc.vector.tensor_copy(bstage[:, i, :], ps_b[:])
                    nc.sync.dma_start(bdram_v[:, t0:t0 + k, :], bstage[:, :k, :])
                    t0 += k

                # ---- phase 1b: own tiles: keep x, x^T, and A = x@W1a + b1
                ownx_v = ownx_d.rearrange("(n p) d -> p n d", p=128)
                nc.sync.dma_start(x_own[:], ownx_v[:])
                for b in range(BLOCKS):
                    ps_t = ps1.tile([128, 128], f32, tag="ps_t")
                    nc.tensor.transpose(ps_t[:], x_own[:, b, :], ident[:])
                    nc.scalar.copy(xT_own[:, b, :], ps_t[:])
                    ps_a = ps1.tile([128, 128], f32, tag="ps_b")
                    nc.tensor.matmul(ps_a[:], xT_own[:, b, :], w1a[:], start=True, stop=False)
                    nc.tensor.matmul(ps_a[:], ones_row[:], b1[:], start=False, stop=True)
                    nc.vector.tensor_copy(a_own[:, b, :], ps_a[:])

            # ---- phase 2+3: gather, Hsum, update net
            with tc.tile_pool(name="p2", bufs=2) as p2, \
                 tc.tile_pool(name="psh", bufs=2, space="PSUM") as psh, \
                 tc.tile_pool(name="ps2", bufs=1, space="PSUM") as ps2:
                for b in range(BLOCKS):
                    g = p2.tile([128, M, D], f32, tag="g")
                    nc.gpsimd.dma_gather(
                        g[:], bdram[:], idx_sb[:, b * 256:(b + 1) * 256],
                        M * 128, M * 128, D, single_packet=False,
                    )
                    nc.vector.tensor_tensor(
                        out=g[:], in0=g[:],
                        in1=a_own[:, b:b + 1, :].to_broadcast([128, M, D]),
                        op=mybir.AluOpType.add,
                    )
                    nc.vector.tensor_scalar_max(g[:], g[:], 0.0)

                    ps_h = psh.tile([128, 128], f32, tag="ps_h")
                    for m in range(M):
                        nc.tensor.matmul(ps_h[:], ident[:], g[:, m, :],
                                         start=(m == 0), stop=(m == M - 1))
                    hs = p2.tile([128, 128], f32, tag="hs")
                    nc.scalar.copy(hs[:], ps_h[:])

                    ps_ht = ps2.tile([128, 128], f32, tag="ps_ht")
                    nc.tensor.transpose(ps_ht[:], hs[:], ident[:])
                    hst = p2.tile([128, 128], f32, tag="hst")
                    nc.scalar.copy(hst[:], ps_ht[:])

                    ps_pre = ps2.tile([128, 128], f32, tag="ps_pre")
                    nc.tensor.matmul(ps_pre[:], xT_own[:, b, :], u1a[:], start=True, stop=False)
                    nc.tensor.matmul(ps_pre[:], hst[:], w2u[:], start=False, stop=False)
                    nc.tensor.matmul(ps_pre[:], ones_row[:], biasu[:], start=False, stop=True)
                    u = p2.tile([128, 128], f32, tag="u")
                    nc.vector.tensor_scalar_max(u[:], ps_pre[:], 0.0)

                    ps_ut = ps2.tile([128, 128], f32, tag="ps_ut")
                    nc.tensor.transpose(ps_ut[:], u[:], ident[:])
                    ut = p2.tile([128, 128], f32, tag="ut")
                    nc.scalar.copy(ut[:], ps_ut[:])

                    ps_o = ps2.tile([128, 128], f32, tag="ps_o")
                    nc.tensor.matmul(ps_o[:], ut[:], uw2[:], start=True, stop=False)
                    nc.tensor.matmul(ps_o[:], ones_row[:], bu2[:], start=False, stop=False)
                    nc.tensor.matmul(ps_o[:], ident[:], x_own[:, b, :], start=False, stop=True)
                    nc.vector.tensor_scalar_max(ostage[:, b, :], ps_o[:], 0.0)

                nc.sync.dma_start(out_v[:], ostage[:])

    nc.compile()
    nc.m = get_hw_module(nc.m)
    return nc


def get_nc():
    if "nc" not in _CACHE:
        _CACHE["nc"] = _build_nc()
    return _CACHE["nc"]


def make_in_maps(atom_features, nbr_indices,
                 msg_W1, msg_b1, msg_W2, msg_b2,
                 upd_W1, upd_b1, upd_W2, upd_b2):
    atom_features = np.ascontiguousarray(np.asarray(atom_features, dtype=np.float32))
    nbr = np.asarray(nbr_indices)

    atoms = np.zeros((N_PAD, D), dtype=np.float32)
    atoms[:N_ATOMS] = atom_features

    idx = np.zeros((N_PAD, M), dtype=np.int16)
    idx[:N_ATOMS] = nbr.astype(np.int16)
    # per core/block: logical order j = m*128 + p; wrapped [16, 256] then
    # replicated to 128 partitions: unwrapped[j] = tile[j % 16, j // 16]
    idx = idx.reshape(N_CORES, BLOCKS, 128, M)
    idx = idx.transpose(0, 1, 3, 2)                 # [core, blk, m, p] -> L[j]
    idx = idx.reshape(N_CORES, BLOCKS * M * 128 // 16, 16)
    idx = idx.transpose(0, 2, 1)                    # [core, 16, 6400]
    idx16 = np.tile(idx, (1, 8, 1))                 # [core, 128, 6400]
    idx16 = np.ascontiguousarray(idx16)

    w = {
        "w1a": np.ascontiguousarray(np.asarray(msg_W1[:D], dtype=np.float32)),
        "w1b": np.ascontiguousarray(np.asarray(msg_W1[D:], dtype=np.float32)),
        "b1": np.asarray(msg_b1, dtype=np.float32).reshape(1, D),
        "w2": np.ascontiguousarray(np.asarray(msg_W2, dtype=np.float32)),
        "b2c": np.asarray(msg_b2, dtype=np.float32).reshape(D, 1),
        "u1a": np.ascontiguousarray(np.asarray(upd_W1[:D], dtype=np.float32)),
        "u1b": np.ascontiguousarray(np.asarray(upd_W1[D:], dtype=np.float32)),
        "bu1": np.asarray(upd_b1, dtype=np.float32).reshape(1, D),
        "uw2": np.ascontiguousarray(np.asarray(upd_W2, dtype=np.float32)),
        "bu2": np.asarray(upd_b2, dtype=np.float32).reshape(1, D),
    }

    in_maps = []
    for c in range(N_CORES):
        m = {
            "atoms": atoms,
            "own_x": atoms[c * OWN:(c + 1) * OWN],
            "idx16": idx16[c],
        }
        m.update(w)
        in_maps.append(m)
    return in_maps


def kernel(atom_features, nbr_features, nbr_indices,
           msg_W1, msg_b1, msg_W2, msg_b2,
           upd_W1, upd_b1, upd_W2, upd_b2):
    global last_results
    from concourse.bass_utils import run_bass_kernel_spmd

    nc = get_nc()
    in_maps = make_in_maps(atom_features, nbr_indices,
                           msg_W1, msg_b1, msg_W2, msg_b2,
                           upd_W1, upd_b1, upd_W2, upd_b2)
    res = run_bass_kernel_spmd(nc, in_maps, core_ids=list(range(N_CORES)))
    last_results = res
    out = np.concatenate([res.results[c]["out"] for c in range(N_CORES)], axis=0)
    return out[:N_ATOMS]


# revision 3
# speedup vs baseline: 1.4713x; 1.4713x over previous
"""AtomicConvLayer (GNN message passing) on 8 Trainium2 NeuronCores.

Reference computation (per atom i, neighbors j = nbr[i, 0..31]):
    h_ij   = relu(x_i @ W1a + x_j @ W1b + b1)         (msg_W1 split in two)
    agg_i  = sum_j (h_ij @ W2 + b2)
    u_i    = relu(x_i @ U1a + agg_i @ U1b + bu1)
    out_i  = relu(x_i + u_i @ UW2 + bu2)

Algebraic restructuring used here (exact in exact arithmetic):
    B      = X @ W1b                (25600x128 table, computed per core)
    A_i    = x_i @ W1a + b1
    Hsum_i = sum_j relu(A_i + B[nbr_ij])             <- only gather B rows
    u_i    = relu(x_i @ U1a + Hsum_i @ (W2 @ U1b) + (bu1 + 32*b2 @ U1b))
    out_i  = relu(x_i + u_i @ UW2 + bu2)

Sharding: data-parallel over atoms. Each core owns 3200 consecutive atoms
(25000 padded to 25600), holds the full atom table, computes the full B
table locally (12.8 MB, cheaper than cross-core gathers), then gathers its
own 3200*32 neighbor rows from B with dma_gather.
"""

import sys

sys.path.insert(0, "/opt/trn_rl_repo")

import numpy as np

N_ATOMS = 25000
N_PAD = 25600          # 8 cores x 3200
D = 128
M = 32                 # neighbors per atom
N_CORES = 8
OWN = N_PAD // N_CORES          # 3200 atoms per core
BLOCKS = OWN // 128             # 25 blocks of 128 atoms per core
TILES = N_PAD // 128            # 200 tiles in the full table
LOAD_CHUNK = 16                 # tiles per phase-1 DMA

_CACHE = {}
last_results = None


def _build_nc():
    import concourse.bacc as bacc
    import concourse.mybir as mybir
    import concourse.tile as tile
    from concourse.bass_interp import get_hw_module
    from concourse.masks import make_identity

    f32 = mybir.dt.float32
    nc = bacc.Bacc("TRN2", target_bir_lowering=False, debug=False,
                   num_swdge_queues=4)

    atoms_d = nc.dram_tensor("atoms", [N_PAD, D], f32, kind="ExternalInput")
    ownx_d = nc.dram_tensor("own_x", [OWN, D], f32, kind="ExternalInput")
    idx_d = nc.dram_tensor("idx16", [128, BLOCKS * 256], mybir.dt.int16, kind="ExternalInput")
    w1a_d = nc.dram_tensor("w1a", [D, D], f32, kind="ExternalInput")
    w1b_d = nc.dram_tensor("w1b", [D, D], f32, kind="ExternalInput")
    b1_d = nc.dram_tensor("b1", [1, D], f32, kind="ExternalInput")
    w2_d = nc.dram_tensor("w2", [D, D], f32, kind="ExternalInput")
    b2c_d = nc.dram_tensor("b2c", [D, 1], f32, kind="ExternalInput")
    u1a_d = nc.dram_tensor("u1a", [D, D], f32, kind="ExternalInput")
    u1b_d = nc.dram_tensor("u1b", [D, D], f32, kind="ExternalInput")
    bu1_d = nc.dram_tensor("bu1", [1, D], f32, kind="ExternalInput")
    uw2_d = nc.dram_tensor("uw2", [D, D], f32, kind="ExternalInput")
    bu2_d = nc.dram_tensor("bu2", [1, D], f32, kind="ExternalInput")
    out_d = nc.dram_tensor("out", [OWN, D], f32, kind="ExternalOutput")

    atoms_v = atoms_d.rearrange("(n p) d -> p n d", p=128)   # [128, 200, 128]
    out_v = out_d.rearrange("(n p) d -> p n d", p=128)       # [128, 25, 128]

    with tile.TileContext(nc) as tc:
        with (
            tc.tile_pool(name="persist", bufs=1) as per,
            tc.tile_pool(name="dram", bufs=1, space="DRAM") as dram,
        ):
            ident = per.tile([128, 128], f32)
            make_identity(nc, ident[:])
            ones_row = per.tile([1, 128], f32)
            nc.gpsimd.memset(ones_row[:], 1.0)

            w1a = per.tile([D, D], f32)
            w1b = per.tile([D, D], f32)
            b1 = per.tile([1, D], f32)
            w2 = per.tile([D, D], f32)
            b2c = per.tile([D, 1], f32)
            u1a = per.tile([D, D], f32)
            u1b = per.tile([D, D], f32)
            bu1 = per.tile([1, D], f32)
            uw2 = per.tile([D, D], f32)
            bu2 = per.tile([1, D], f32)
            idx_sb = per.tile([128, BLOCKS * 256], mybir.dt.int16)
            for sb, d in [(w1a, w1a_d), (w1b, w1b_d), (b1, b1_d), (w2, w2_d),
                          (b2c, b2c_d), (u1a, u1a_d), (u1b, u1b_d), (bu1, bu1_d),
                          (uw2, uw2_d), (bu2, bu2_d), (idx_sb, idx_d)]:
                nc.sync.dma_start(sb[:], d[:])

            x_own = per.tile([128, BLOCKS, D], f32)
            xT_own = per.tile([128, BLOCKS, D], f32)
            a_own = per.tile([128, BLOCKS, D], f32)
            ostage = per.tile([128, BLOCKS, D], f32)
            w2u = per.tile([D, D], f32)
            biasu = per.tile([1, D], f32)

            bdram = dram.tile([N_PAD, D], f32)
            bdram_v = bdram[:].rearrange("(n p) d -> p n d", p=128)

            # ---- weight folds: w2u = W2 @ U1b ; biasu = bu1 + 32*b2 @ U1b
            with tc.tile_pool(name="ps0", bufs=1, space="PSUM") as ps0:
                ps_wt = ps0.tile([128, 128], f32)
                nc.tensor.transpose(ps_wt[:], w2[:], ident[:])
                w2t = per.tile([D, D], f32)
                nc.vector.tensor_copy(w2t[:], ps_wt[:])
                ps_w2u = ps0.tile([128, 128], f32)
                nc.tensor.matmul(ps_w2u[:], w2t[:], u1b[:], start=True, stop=True)
                nc.vector.tensor_copy(w2u[:], ps_w2u[:])

                b2s = per.tile([D, 1], f32)
                nc.vector.tensor_scalar_mul(b2s[:], b2c[:], float(M))
                ps_c = ps0.tile([1, 128], f32)
                nc.tensor.matmul(ps_c[:], b2s[:], u1b[:], start=True, stop=True)
                nc.vector.tensor_tensor(out=biasu[:], in0=ps_c[:], in1=bu1[:],
                                        op=mybir.AluOpType.add)

            # ---- phase 1: B = atoms @ W1b  -> bdram
            with tc.tile_pool(name="p1", bufs=2) as p1, \
                 tc.tile_pool(name="ps1", bufs=2, space="PSUM") as ps1:
                t0 = 0
                while t0 < TILES:
                    k = min(LOAD_CHUNK, TILES - t0)
                    xin = p1.tile([128, LOAD_CHUNK, D], f32, tag="xin")
                    nc.sync.dma_start(xin[:, :k, :], atoms_v[:, t0:t0 + k, :])
                    bstage = p1.tile([128, LOAD_CHUNK, D], f32, tag="bstage")
                    for i in range(k):
                        ps_t = ps1.tile([128, 128], f32, tag="ps_t")
                        nc.tensor.transpose(ps_t[:], xin[:, i, :], ident[:])
                        xt = p1.tile([128, 128], f32, tag="xt")
                        nc.scalar.copy(xt[:], ps_t[:])
                        ps_b = ps1.tile([128, 128], f32, tag="ps_b")
                        nc.tensor.matmul(ps_b[:], xt[:], w1b[:], start=True, stop=True)
                        nc.vector.tensor_copy(bstage[:, i, :], ps_b[:])
                    nc.sync.dma_start(bdram_v[:, t0:t0 + k, :], bstage[:, :k, :])
                    t0 += k

                # ---- phase 1b: own tiles: keep x, x^T, and A = x@W1a + b1
                ownx_v = ownx_d.rearrange("(n p) d -> p n d", p=128)
                nc.sync.dma_start(x_own[:], ownx_v[:])
                for b in range(BLOCKS):
                    ps_t = ps1.tile([128, 128], f32, tag="ps_t")
                    nc.tensor.transpose(ps_t[:], x_own[:, b, :], ident[:])
                    nc.scalar.copy(xT_own[:, b, :], ps_t[:])
                    ps_a = ps1.tile([128, 128], f32, tag="ps_b")
                    nc.tensor.matmul(ps_a[:], xT_own[:, b, :], w1a[:], start=True, stop=False)
                    nc.tensor.matmul(ps_a[:], ones_row[:], b1[:], start=False, stop=True)
                    nc.vector.tensor_copy(a_own[:, b, :], ps_a[:])

            # ---- phase 2+3: gather, Hsum, update net
            with tc.tile_pool(name="p2", bufs=2) as p2, \
                 tc.tile_pool(name="psh", bufs=2, space="PSUM") as psh, \
                 tc.tile_pool(name="ps2", bufs=1, space="PSUM") as ps2:
                for b in range(BLOCKS):
                    g = p2.tile([128, M, D], f32, tag="g")
                    nc.gpsimd.dma_gather(
                        g[:], bdram[:], idx_sb[:, b * 256:(b + 1) * 256],
                        M * 128, M * 128, D, single_packet=False,
                        queue_num=b % 4,
                    )
                    nc.vector.tensor_tensor(
                        out=g[:], in0=g[:],
                        in1=a_own[:, b:b + 1, :].to_broadcast([128, M, D]),
                        op=mybir.AluOpType.add,
                    )
                    nc.vector.tensor_scalar_max(g[:], g[:], 0.0)

                    ps_h = psh.tile([128, 128], f32, tag="ps_h")
                    for m in range(M):
                        nc.tensor.matmul(ps_h[:], ident[:], g[:, m, :],
                                         start=(m == 0), stop=(m == M - 1))
                    hs = p2.tile([128, 128], f32, tag="hs")
                    nc.scalar.copy(hs[:], ps_h[:])

                    ps_ht = ps2.tile([128, 128], f32, tag="ps_ht")
                    nc.tensor.transpose(ps_ht[:], hs[:], ident[:])
                    hst = p2.tile([128, 128], f32, tag="hst")
                    nc.scalar.copy(hst[:], ps_ht[:])

                    ps_pre = ps2.tile([128, 128], f32, tag="ps_pre")
                    nc.tensor.matmul(ps_pre[:], xT_own[:, b, :], u1a[:], start=True, stop=False)
                    nc.tensor.matmul(ps_pre[:], hst[:], w2u[:], start=False, stop=False)
                    nc.tensor.matmul(ps_pre[:], ones_row[:], biasu[:], start=False, stop=True)
                    u = p2.tile([128, 128], f32, tag="u")
                    nc.vector.tensor_scalar_max(u[:], ps_pre[:], 0.0)

                    ps_ut = ps2.tile([128, 128], f32, tag="ps_ut")
                    nc.tensor.transpose(ps_ut[:], u[:], ident[:])
                    ut = p2.tile([128, 128], f32, tag="ut")
                    nc.scalar.copy(ut[:], ps_ut[:])

                    ps_o = ps2.tile([128, 128], f32, tag="ps_o")
                    nc.tensor.matmul(ps_o[:], ut[:], uw2[:], start=True, stop=False)
                    nc.tensor.matmul(ps_o[:], ones_row[:], bu2[:], start=False, stop=False)
                    nc.tensor.matmul(ps_o[:], ident[:], x_own[:, b, :], start=False, stop=True)
                    nc.vector.tensor_scalar_max(ostage[:, b, :], ps_o[:], 0.0)

                nc.sync.dma_start(out_v[:], ostage[:])

    nc.compile()
    nc.m = get_hw_module(nc.m)
    return nc


def get_nc():
    if "nc" not in _CACHE:
        _CACHE["nc"] = _build_nc()
    return _CACHE["nc"]


def make_in_maps(atom_features, nbr_indices,
                 msg_W1, msg_b1, msg_W2, msg_b2,
                 upd_W1, upd_b1, upd_W2, upd_b2):
    atom_features = np.ascontiguousarray(np.asarray(atom_features, dtype=np.float32))
    nbr = np.asarray(nbr_indices)

    atoms = np.zeros((N_PAD, D), dtype=np.float32)
    atoms[:N_ATOMS] = atom_features

    idx = np.zeros((N_PAD, M), dtype=np.int16)
    idx[:N_ATOMS] = nbr.astype(np.int16)
    # per core/block: logical order j = m*128 + p; wrapped [16, 256] then
    # replicated to 128 partitions: unwrapped[j] = tile[j % 16, j // 16]
    idx = idx.reshape(N_CORES, BLOCKS, 128, M)
    idx = idx.transpose(0, 1, 3, 2)                 # [core, blk, m, p] -> L[j]
    idx = idx.reshape(N_CORES, BLOCKS * M * 128 // 16, 16)
    idx = idx.transpose(0, 2, 1)                    # [core, 16, 6400]
    idx16 = np.tile(idx, (1, 8, 1))                 # [core, 128, 6400]
    idx16 = np.ascontiguousarray(idx16)

    w = {
        "w1a": np.ascontiguousarray(np.asarray(msg_W1[:D], dtype=np.float32)),
        "w1b": np.ascontiguousarray(np.asarray(msg_W1[D:], dtype=np.float32)),
        "b1": np.asarray(msg_b1, dtype=np.float32).reshape(1, D),
        "w2": np.ascontiguousarray(np.asarray(msg_W2, dtype=np.float32)),
        "b2c": np.asarray(msg_b2, dtype=np.float32).reshape(D, 1),
        "u1a": np.ascontiguousarray(np.asarray(upd_W1[:D], dtype=np.float32)),
        "u1b": np.ascontiguousarray(np.asarray(upd_W1[D:], dtype=np.float32)),
        "bu1": np.asarray(upd_b1, dtype=np.float32).reshape(1, D),
        "uw2": np.ascontiguousarray(np.asarray(upd_W2, dtype=np.float32)),
        "bu2": np.asarray(upd_b2, dtype=np.float32).reshape(1, D),
    }

    in_maps = []
    for c in range(N_CORES):
        m = {
            "atoms": atoms,
            "own_x": atoms[c * OWN:(c + 1) * OWN],
            "idx16": idx16[c],
        }
        m.update(w)
        in_maps.append(m)
    return in_maps


def kernel(atom_features, nbr_features, nbr_indices,
           msg_W1, msg_b1, msg_W2, msg_b2,
           upd_W1, upd_b1, upd_W2, upd_b2):
    global last_results
    from concourse.bass_utils import run_bass_kernel_spmd

    nc = get_nc()
    in_maps = make_in_maps(atom_features, nbr_indices,
                           msg_W1, msg_b1, msg_W2, msg_b2,
                           upd_W1, upd_b1, upd_W2, upd_b2)
    res = run_bass_kernel_spmd(nc, in_maps, core_ids=list(range(N_CORES)))
    last_results = res
    out = np.concatenate([res.results[c]["out"] for c in range(N_CORES)], axis=0)
    return out[:N_ATOMS]


# revision 4
# speedup vs baseline: 1.6587x; 1.1273x over previous
"""AtomicConvLayer (GNN message passing) on 8 Trainium2 NeuronCores.

Reference computation (per atom i, neighbors j = nbr[i, 0..31]):
    h_ij   = relu(x_i @ W1a + x_j @ W1b + b1)         (msg_W1 split in two)
    agg_i  = sum_j (h_ij @ W2 + b2)
    u_i    = relu(x_i @ U1a + agg_i @ U1b + bu1)
    out_i  = relu(x_i + u_i @ UW2 + bu2)

Algebraic restructuring used here (exact in exact arithmetic):
    B      = X @ W1b                (25600x128 table, computed per core)
    A_i    = x_i @ W1a + b1
    Hsum_i = sum_j relu(A_i + B[nbr_ij])             <- only gather B rows
    u_i    = relu(x_i @ U1a + Hsum_i @ (W2 @ U1b) + (bu1 + 32*b2 @ U1b))
    out_i  = relu(x_i + u_i @ UW2 + bu2)

Sharding: data-parallel over atoms. Each core owns 3200 consecutive atoms
(25000 padded to 25600), holds the full atom table, computes the full B
table locally (12.8 MB, cheaper than cross-core gathers), then gathers its
own 3200*32 neighbor rows from B with dma_gather.
"""

import sys

sys.path.insert(0, "/opt/trn_rl_repo")

import numpy as np

N_ATOMS = 25000
N_PAD = 25600          # 8 cores x 3200
D = 128
M = 32                 # neighbors per atom
N_CORES = 8
OWN = N_PAD // N_CORES          # 3200 atoms per core
BLOCKS = OWN // 128             # 25 blocks of 128 atoms per core
TILES = N_PAD // 128            # 200 tiles in the full table
LOAD_CHUNK = 16                 # tiles per phase-1 DMA

_CACHE = {}
last_results = None


def _build_nc():
    import concourse.bacc as bacc
    import concourse.mybir as mybir
    import concourse.tile as tile
    from concourse.bass_interp import get_hw_module
    from concourse.masks import make_identity

    f32 = mybir.dt.float32
    nc = bacc.Bacc("TRN2", target_bir_lowering=False, debug=False,
                   num_swdge_queues=4)

    atoms_d = nc.dram_tensor("atoms", [N_PAD, D], f32, kind="ExternalInput")
    ownx_d = nc.dram_tensor("own_x", [OWN, D], f32, kind="ExternalInput")
    idx_d = nc.dram_tensor("idx16", [128, BLOCKS * 256], mybir.dt.int16, kind="ExternalInput")
    w1a_d = nc.dram_tensor("w1a", [D, D], f32, kind="ExternalInput")
    w1b_d = nc.dram_tensor("w1b", [D, D], f32, kind="ExternalInput")
    b1_d = nc.dram_tensor("b1", [1, D], f32, kind="ExternalInput")
    w2_d = nc.dram_tensor("w2", [D, D], f32, kind="ExternalInput")
    b2c_d = nc.dram_tensor("b2c", [D, 1], f32, kind="ExternalInput")
    u1a_d = nc.dram_tensor("u1a", [D, D], f32, kind="ExternalInput")
    u1b_d = nc.dram_tensor("u1b", [D, D], f32, kind="ExternalInput")
    bu1_d = nc.dram_tensor("bu1", [1, D], f32, kind="ExternalInput")
    uw2_d = nc.dram_tensor("uw2", [D, D], f32, kind="ExternalInput")
    bu2_d = nc.dram_tensor("bu2", [1, D], f32, kind="ExternalInput")
    out_d = nc.dram_tensor("out", [OWN, D], f32, kind="ExternalOutput")

    atoms_v = atoms_d.rearrange("(n p) d -> p n d", p=128)   # [128, 200, 128]
    out_v = out_d.rearrange("(n p) d -> p n d", p=128)       # [128, 25, 128]

    with tile.TileContext(nc) as tc:
        with (
            tc.tile_pool(name="persist", bufs=1) as per,
            tc.tile_pool(name="dram", bufs=1, space="DRAM") as dram,
        ):
            ident = per.tile([128, 128], f32)
            make_identity(nc, ident[:])
            ones_row = per.tile([1, 128], f32)
            nc.gpsimd.memset(ones_row[:], 1.0)

            w1a = per.tile([D, D], f32)
            w1b = per.tile([D, D], f32)
            b1 = per.tile([1, D], f32)
            w2 = per.tile([D, D], f32)
            b2c = per.tile([D, 1], f32)
            u1a = per.tile([D, D], f32)
            u1b = per.tile([D, D], f32)
            bu1 = per.tile([1, D], f32)
            uw2 = per.tile([D, D], f32)
            bu2 = per.tile([1, D], f32)
            idx_sb = per.tile([128, BLOCKS * 256], mybir.dt.int16)
            for sb, d in [(w1a, w1a_d), (w1b, w1b_d), (b1, b1_d), (w2, w2_d),
                          (b2c, b2c_d), (u1a, u1a_d), (u1b, u1b_d), (bu1, bu1_d),
                          (uw2, uw2_d), (bu2, bu2_d), (idx_sb, idx_d)]:
                nc.sync.dma_start(sb[:], d[:])

            x_own = per.tile([128, BLOCKS, D], f32)
            xT_own = per.tile([128, BLOCKS, D], f32)
            a_own = per.tile([128, BLOCKS, D], f32)
            ostage = per.tile([128, BLOCKS, D], f32)
            w2u = per.tile([D, D], f32)
            biasu = per.tile([1, D], f32)

            bdram = dram.tile([N_PAD, D], f32)
            bdram_v = bdram[:].rearrange("(n p) d -> p n d", p=128)

            # ---- weight folds: w2u = W2 @ U1b ; biasu = bu1 + 32*b2 @ U1b
            with tc.tile_pool(name="ps0", bufs=1, space="PSUM") as ps0:
                ps_wt = ps0.tile([128, 128], f32)
                nc.tensor.transpose(ps_wt[:], w2[:], ident[:])
                w2t = per.tile([D, D], f32)
                nc.vector.tensor_copy(w2t[:], ps_wt[:])
                ps_w2u = ps0.tile([128, 128], f32)
                nc.tensor.matmul(ps_w2u[:], w2t[:], u1b[:], start=True, stop=True)
                nc.vector.tensor_copy(w2u[:], ps_w2u[:])

                b2s = per.tile([D, 1], f32)
                nc.vector.tensor_scalar_mul(b2s[:], b2c[:], float(M))
                ps_c = ps0.tile([1, 128], f32)
                nc.tensor.matmul(ps_c[:], b2s[:], u1b[:], start=True, stop=True)
                nc.vector.tensor_tensor(out=biasu[:], in0=ps_c[:], in1=bu1[:],
                                        op=mybir.AluOpType.add)

            # ---- phase 1: B = atoms @ W1b  -> bdram
            with tc.tile_pool(name="p1", bufs=2) as p1, \
                 tc.tile_pool(name="ps1", bufs=2, space="PSUM") as ps1:
                t0 = 0
                while t0 < TILES:
                    k = min(LOAD_CHUNK, TILES - t0)
                    xin = p1.tile([128, LOAD_CHUNK, D], f32, tag="xin")
                    nc.sync.dma_start(xin[:, :k, :], atoms_v[:, t0:t0 + k, :])
                    bstage = p1.tile([128, LOAD_CHUNK, D], f32, tag="bstage")
                    for i in range(k):
                        ps_t = ps1.tile([128, 128], f32, tag="ps_t")
                        nc.tensor.transpose(ps_t[:], xin[:, i, :], ident[:])
                        xt = p1.tile([128, 128], f32, tag="xt")
                        nc.scalar.copy(xt[:], ps_t[:])
                        ps_b = ps1.tile([128, 128], f32, tag="ps_b")
                        nc.tensor.matmul(ps_b[:], xt[:], w1b[:], start=True, stop=True)
                        nc.vector.tensor_copy(bstage[:, i, :], ps_b[:])
                    nc.sync.dma_start(bdram_v[:, t0:t0 + k, :], bstage[:, :k, :])
                    t0 += k

                # ---- phase 1b: own tiles: keep x, x^T, and A = x@W1a + b1
                ownx_v = ownx_d.rearrange("(n p) d -> p n d", p=128)
                nc.sync.dma_start(x_own[:], ownx_v[:])
                for b in range(BLOCKS):
                    ps_t = ps1.tile([128, 128], f32, tag="ps_t")
                    nc.tensor.transpose(ps_t[:], x_own[:, b, :], ident[:])
                    nc.scalar.copy(xT_own[:, b, :], ps_t[:])
                    ps_a = ps1.tile([128, 128], f32, tag="ps_b")
                    nc.tensor.matmul(ps_a[:], xT_own[:, b, :], w1a[:], start=True, stop=False)
                    nc.tensor.matmul(ps_a[:], ones_row[:], b1[:], start=False, stop=True)
                    nc.vector.tensor_copy(a_own[:, b, :], ps_a[:])

            # ---- phase 2+3: gather, Hsum, update net
            with tc.tile_pool(name="p2", bufs=3) as p2, \
                 tc.tile_pool(name="ps2", bufs=2, space="PSUM") as ps2:
                for b in range(BLOCKS):
                    g = p2.tile([128, M, D], f32, tag="g")
                    nc.gpsimd.dma_gather(
                        g[:], bdram[:], idx_sb[:, b * 256:(b + 1) * 256],
                        M * 128, M * 128, D, single_packet=False,
                        queue_num=b % 4,
                    )
                    nc.vector.tensor_tensor(
                        out=g[:], in0=g[:],
                        in1=a_own[:, b:b + 1, :].to_broadcast([128, M, D]),
                        op=mybir.AluOpType.add,
                    )
                    nc.scalar.activation(g[:], g[:],
                                         mybir.ActivationFunctionType.Relu)
                    # Hsum = sum over m: strided-AP reduce along the m axis
                    hs = p2.tile([128, 128], f32, tag="hs")
                    nc.vector.reduce_sum(
                        out=hs[:], in_=g[:].rearrange("p m f -> p f m"),
                        axis=mybir.AxisListType.X)

                    ps_ht = ps2.tile([128, 128], f32, tag="ps_ht")
                    nc.tensor.transpose(ps_ht[:], hs[:], ident[:])
                    hst = p2.tile([128, 128], f32, tag="hst")
                    nc.scalar.copy(hst[:], ps_ht[:])

                    ps_pre = ps2.tile([128, 128], f32, tag="ps_pre")
                    nc.tensor.matmul(ps_pre[:], xT_own[:, b, :], u1a[:], start=True, stop=False)
                    nc.tensor.matmul(ps_pre[:], hst[:], w2u[:], start=False, stop=False)
                    nc.tensor.matmul(ps_pre[:], ones_row[:], biasu[:], start=False, stop=True)
                    u = p2.tile([128, 128], f32, tag="u")
                    nc.vector.tensor_scalar_max(u[:], ps_pre[:], 0.0)

                    ps_ut = ps2.tile([128, 128], f32, tag="ps_ut")
                    nc.tensor.transpose(ps_ut[:], u[:], ident[:])
                    ut = p2.tile([128, 128], f32, tag="ut")
                    nc.scalar.copy(ut[:], ps_ut[:])

                    ps_o = ps2.tile([128, 128], f32, tag="ps_o")
                    nc.tensor.matmul(ps_o[:], ut[:], uw2[:], start=True, stop=False)
                    nc.tensor.matmul(ps_o[:], ones_row[:], bu2[:], start=False, stop=False)
                    nc.tensor.matmul(ps_o[:], ident[:], x_own[:, b, :], start=False, stop=True)
                    nc.vector.tensor_scalar_max(ostage[:, b, :], ps_o[:], 0.0)

                nc.sync.dma_start(out_v[:], ostage[:])

    nc.compile()
    nc.m = get_hw_module(nc.m)
    return nc


def get_nc():
    if "nc" not in _CACHE:
        _CACHE["nc"] = _build_nc()
    return _CACHE["nc"]


def make_in_maps(atom_features, nbr_indices,
                 msg_W1, msg_b1, msg_W2, msg_b2,
                 upd_W1, upd_b1, upd_W2, upd_b2):
    atom_features = np.ascontiguousarray(np.asarray(atom_features, dtype=np.float32))
    nbr = np.asarray(nbr_indices)

    atoms = np.zeros((N_PAD, D), dtype=np.float32)
    atoms[:N_ATOMS] = atom_features

    idx = np.zeros((N_PAD, M), dtype=np.int16)
    idx[:N_ATOMS] = nbr.astype(np.int16)
    # per core/block: logical order j = m*128 + p; wrapped [16, 256] then
    # replicated to 128 partitions: unwrapped[j] = tile[j % 16, j // 16]
    idx = idx.reshape(N_CORES, BLOCKS, 128, M)
    idx = idx.transpose(0, 1, 3, 2)                 # [core, blk, m, p] -> L[j]
    idx = idx.reshape(N_CORES, BLOCKS * M * 128 // 16, 16)
    idx = idx.transpose(0, 2, 1)                    # [core, 16, 6400]
    idx16 = np.tile(idx, (1, 8, 1))                 # [core, 128, 6400]
    idx16 = np.ascontiguousarray(idx16)

    w = {
        "w1a": np.ascontiguousarray(np.asarray(msg_W1[:D], dtype=np.float32)),
        "w1b": np.ascontiguousarray(np.asarray(msg_W1[D:], dtype=np.float32)),
        "b1": np.asarray(msg_b1, dtype=np.float32).reshape(1, D),
        "w2": np.ascontiguousarray(np.asarray(msg_W2, dtype=np.float32)),
        "b2c": np.asarray(msg_b2, dtype=np.float32).reshape(D, 1),
        "u1a": np.ascontiguousarray(np.asarray(upd_W1[:D], dtype=np.float32)),
        "u1b": np.ascontiguousarray(np.asarray(upd_W1[D:], dtype=np.float32)),
        "bu1": np.asarray(upd_b1, dtype=np.float32).reshape(1, D),
        "uw2": np.ascontiguousarray(np.asarray(upd_W2, dtype=np.float32)),
        "bu2": np.asarray(upd_b2, dtype=np.float32).reshape(1, D),
    }

    in_maps = []
    for c in range(N_CORES):
        m = {
            "atoms": atoms,
            "own_x": atoms[c * OWN:(c + 1) * OWN],
            "idx16": idx16[c],
        }
        m.update(w)
        in_maps.append(m)
    return in_maps


def kernel(atom_features, nbr_features, nbr_indices,
           msg_W1, msg_b1, msg_W2, msg_b2,
           upd_W1, upd_b1, upd_W2, upd_b2):
    global last_results
    from concourse.bass_utils import run_bass_kernel_spmd

    nc = get_nc()
    in_maps = make_in_maps(atom_features, nbr_indices,
                           msg_W1, msg_b1, msg_W2, msg_b2,
                           upd_W1, upd_b1, upd_W2, upd_b2)
    res = run_bass_kernel_spmd(nc, in_maps, core_ids=list(range(N_CORES)))
    last_results = res
    out = np.concatenate([res.results[c]["out"] for c in range(N_CORES)], axis=0)
    return out[:N_ATOMS]
